# revision 1
# baseline (speedup 1.0000x reference)
"""Causal multi-head attention (B=2, S=2048, D=1024, H=16) on 8 trn2 cores.

Sharding: core c handles heads {2c, 2c+1} of BOTH batches (4 (b,h) pairs).
Per core:
  - project host-pretransposed x_b^T [D, S] (both batches) through the
    core's Wqkv column slice into Q^T/K^T head-pair tiles and V (natural
    layout, with a fused ones-column that makes the AV matmul emit softmax
    denominators),
  - causal attention per (batch, head) in transposed layout: scores^T =
    K Q^T chunks (PE row-tiled head pairs), exp on ScalarE, causal diagonal
    masks via gpsimd affine_select, A^T V on PE,
  - one 8-wide AllToAll redistributes head outputs so core c holds ALL 16
    heads of batch c//4 for sequence quarter c%4,
  - local projection through the full Wout emits final rows
    512*(c%4) .. +512 of batch c//4.
Host assembles the 8 [512, 1024] shards into (2, 2048, 1024).

Matmuls run in float32r (TF32-like single-pass PE mode, ~1e-3 rel err,
4x faster than true fp32). The PE rounds f32r inputs internally, so DRAM
inputs are declared float32r and DMA'd with the fast HW-DGE path with no
pre-rounding. Set _USE_F32R = False for full fp32.
"""

import sys

for _p in ("/opt/trn_rl_repo", "/opt/pypackages"):
    if _p not in sys.path:
        sys.path.insert(0, _p)

import numpy as np

import concourse.bass as bass
import concourse.mybir as mybir
import concourse.tile as tile
from concourse import bacc
from concourse.bass_utils import run_bass_kernel_spmd

B = 2
S = 2048
D = 1024
H = 16
DH = 64
NCORES = 8
SB = 512           # q block (matmul moving dim)
KC = 128           # k chunk (contraction tile)
NSB = S // SB      # 4 q-blocks
NKC = S // KC      # 16 k-chunks
NDC = D // KC      # 8 contraction chunks for the projections

_USE_F32R = True

_compiled = None


def _build():
    f32 = mybir.dt.float32
    bf16 = mybir.dt.bfloat16
    fr = mybir.dt.float32r if _USE_F32R else f32
    nc = bacc.Bacc(None, target_bir_lowering=False)

    # host-blocked inputs: every [128, N] tile is contiguous in DRAM.
    # Matmul inputs are declared float32r: same 4-byte data, PE rounds
    # internally, and plain (non-casting) sync DMA is allowed.
    xt = nc.declare_dram_parameter("xt", [B, NSB, NDC, KC, SB], fr, isOutput=False)
    wqk = nc.declare_dram_parameter("wqk", [NDC, KC, 2 * KC], fr, isOutput=False)
    wv = nc.declare_dram_parameter("wv", [NDC, KC, 2 * KC], fr, isOutput=False)
    wout = nc.declare_dram_parameter("wout", [NDC, KC, D], fr, isOutput=False)
    bqk = nc.declare_dram_parameter("bqk", [2 * KC], f32, isOutput=False)
    bv = nc.declare_dram_parameter("bv", [2 * DH], f32, isOutput=False)
    bo = nc.declare_dram_parameter("bo", [D], f32, isOutput=False)
    vones = nc.declare_dram_parameter("vones", [KC, NKC], fr, isOutput=False)
    out_ext = nc.declare_dram_parameter("out", [SB, D], f32, isOutput=True)

    # AllToAll staging: block t -> core t gets my heads of batch t//4 for
    # s-quarter t%4.
    a2a_in = nc.dram_tensor("a2a_in", [NCORES, KC, SB], fr)
    a2a_out = nc.dram_tensor("a2a_out", [NCORES, KC, SB], fr)

    with tile.TileContext(nc) as tc:
        with (
            tc.tile_pool(name="qkv", bufs=1) as qkvp,
            tc.tile_pool(name="obuf", bufs=1) as op,
            tc.tile_pool(name="misc", bufs=1) as mp,
            tc.tile_pool(name="evict", bufs=1) as ep,
        ):
            # ---- small constants -----------------------------------------
            bqk_t = [mp.tile([KC, 1], f32, tag=f"bqk{m}", name=f"bqk{m}")
                     for m in range(2)]
            for m in range(2):
                nc.scalar.dma_start(
                    out=bqk_t[m][:],
                    in_=bqk[m * KC:(m + 1) * KC].rearrange("(p o) -> p o", o=1),
                )
            bv_row = mp.tile([1, 2 * DH], f32, tag="bv_row")
            nc.scalar.dma_start(out=bv_row[:], in_=bv.rearrange("(o f) -> o f", o=1))
            bv_bc = mp.tile([KC, 2 * DH], f32, tag="bv_bc")
            nc.gpsimd.partition_broadcast(out_ap=bv_bc[:], in_ap=bv_row[:])
            bo_row = mp.tile([1, D], f32, tag="bo_row")
            nc.scalar.dma_start(out=bo_row[:], in_=bo.rearrange("(o f) -> o f", o=1))
            bo_bc = mp.tile([KC, D], f32, tag="bo_bc")
            nc.gpsimd.partition_broadcast(out_ap=bo_bc[:], in_ap=bo_row[:])

            # ---- persistent activations ----------------------------------
            # pair p = batch p with heads (2c, 2c+1).
            # QQ[p]: rows 0:64 = Q^T of head 2c, rows 64:128 = head 2c+1
            # per-sblk tiles so attention can start before all of the
            # projection finishes (Tile deps are per-tile)
            QQ = [[qkvp.tile([KC, SB], fr, tag=f"QQ{p}_{s}", name=f"QQ{p}_{s}")
                   for s in range(NSB)] for p in range(2)]
            KK = [[qkvp.tile([KC, SB], fr, tag=f"KK{p}_{s}", name=f"KK{p}_{s}")
                   for s in range(NSB)] for p in range(2)]
            # V[2p+hh][s]: [128, 4*65]; chunk sc at cols sc*65..+64; col 64: 1.0
            NCS = SB // KC
            V = [[qkvp.tile([KC, NCS * (DH + 1)], fr, tag=f"V{v}_{s}",
                            name=f"V{v}_{s}")
                  for s in range(NSB)] for v in range(4)]
            vones_sb = mp.tile([KC, NKC], fr, tag="vones_sb")
            nc.scalar.dma_start(out=vones_sb[:], in_=vones[:])
            for v in range(4):
                for s in range(NSB):
                    vv = V[v][s][:].rearrange("p (k c) -> p k c", c=DH + 1)
                    nc.vector.tensor_copy(
                        vv[:, :, DH], vones_sb[:, s * NCS:(s + 1) * NCS])
            # O[p]: rows 0:64 = head 2c out^T (normalized), 64:128 = head 2c+1
            O = [op.tile([KC, S], fr, tag=f"O{p}", name=f"O{p}") for p in range(2)]

            # ---- phase 1: projections ------------------------------------
            with (
                tc.tile_pool(name="pjw", bufs=1) as wp,
                tc.tile_pool(name="xbuf", bufs=24) as xp,
                tc.tile_pool(name="psum_proj", bufs=1, space="PSUM") as pp,
            ):
                wqk_t = [wp.tile([KC, 2 * KC], fr, tag=f"wqk{k}", name=f"wqk{k}")
                         for k in range(NDC)]
                wv_t = [wp.tile([KC, 2 * KC], fr, tag=f"wv{k}", name=f"wv{k}")
                        for k in range(NDC)]
                for k in range(NDC):
                    nc.sync.dma_start(out=wqk_t[k][:], in_=wqk[k])

                for sblk in range(NSB):
                    for bb in range(B):
                        xs = []
                        for k in range(NDC):
                            xtl = xp.tile([KC, SB], fr, tag="xt")
                            eng = nc.sync if k % 2 == 0 else nc.gpsimd
                            eng.dma_start(out=xtl[:], in_=xt[bb, sblk, k])
                            xs.append(xtl)
                        # m-chunk 0 -> QQ[bb], 1 -> KK[bb]
                        for m in range(2):
                            ps = pp.tile([KC, SB], f32, tag="ps_qk", bufs=4)
                            for k in range(NDC):
                                nc.tensor.matmul(
                                    ps[:],
                                    wqk_t[k][:, m * KC:(m + 1) * KC],
                                    xs[k][:],
                                    start=(k == 0),
                                    stop=(k == NDC - 1),
                                )
                            dest = (QQ if m == 0 else KK)[bb][sblk]
                            nc.vector.tensor_scalar_add(
                                dest[:], ps[:], bqk_t[m][:],
                            )
                        if sblk == 0 and bb == 0:
                            # defer Wv loads so the first QK matmuls (which
                            # need only wqk + x) start as early as possible
                            for k in range(NDC):
                                nc.gpsimd.dma_start(out=wv_t[k][:], in_=wv[k])
                        # V natural: lhsT = x^T chunk; rhs = Wv (zero-padded
                        # to N=256 so f32r streams at full rate)
                        for sc in range(SB // KC):
                            ps = pp.tile([KC, 2 * KC], f32, tag="ps_v", bufs=4)
                            for k in range(NDC):
                                nc.tensor.matmul(
                                    ps[:],
                                    xs[k][:, sc * KC:(sc + 1) * KC],
                                    wv_t[k][:],
                                    start=(k == 0),
                                    stop=(k == NDC - 1),
                                )
                            for hh in range(2):
                                nc.vector.tensor_add(
                                    V[2 * bb + hh][sblk][:, sc * (DH + 1):
                                                         sc * (DH + 1) + DH],
                                    ps[:, hh * DH:(hh + 1) * DH],
                                    bv_bc[:, hh * DH:(hh + 1) * DH],
                                )

            # ---- phase 2: attention --------------------------------------
            with (
                tc.tile_pool(name="pbuf", bufs=1) as pb,
                tc.tile_pool(name="psum_att", bufs=1, space="PSUM") as pa,
            ):
                for qblk in range(NSB):
                    nkc = 4 * (qblk + 1)  # causal: k-chunks 0..nkc-1
                    P_all = []
                    for p in range(B):
                        # P[kc]: [128, 1024]; cols hh*512.. hold head hh
                        P = [
                            pb.tile([KC, 2 * SB], fr, tag=f"P{kc}",
                                    name=f"P{kc}_{p}_{qblk}",
                                    bufs=(2 if kc < 11 else 1))
                            for kc in range(nkc)
                        ]
                        P_all.append(P)
                        for kc in range(nkc):
                            d = kc - 4 * qblk
                            # causal: columns < 128*d are fully masked; skip
                            # them in the matmul/exp where the speed holds up
                            c0 = min(KC * max(d, 0), 2 * KC)
                            ps = pa.tile([KC, 2 * SB], f32, tag="ps_s", bufs=3)
                            for hh in range(2):  # row-tiled head pair
                                r0 = hh * DH
                                nc.tensor.matmul(
                                    ps[:, hh * SB + c0:(hh + 1) * SB],
                                    KK[p][kc // 4][r0:r0 + DH,
                                                   (kc % 4) * KC:
                                                   (kc % 4 + 1) * KC],
                                    QQ[p][qblk][r0:r0 + DH, c0:SB],
                                    start=True,
                                    stop=True,
                                )
                            ps3 = ps[:].rearrange("p (h f) -> p h f", h=2)
                            pd3 = P[kc][:].rearrange("p (h f) -> p h f", h=2)
                            e0 = KC * max(d, 0)
                            nc.scalar.activation(
                                pd3[:, :, e0:SB],
                                ps3[:, :, e0:SB],
                                mybir.ActivationFunctionType.Exp,
                                scale=1.0 / float(np.sqrt(DH)),
                            )
                            if d >= 0:  # diagonal chunk: zero where k > q
                                # only columns >= c0 are ever read by the AV
                                # matmul, so mask just that range
                                nc.gpsimd.affine_select(
                                    out=pd3[:, :, c0:SB],
                                    in_=pd3[:, :, c0:SB],
                                    pattern=[[0, 2], [1, SB - c0]],
                                    compare_op=mybir.AluOpType.is_ge,
                                    fill=0.0,
                                    base=c0 - KC * d,
                                    channel_multiplier=-1,
                                )
                    for p in range(B):
                        P = P_all[p]
                        pos = [pa.tile([DH + 1, SB], f32, tag=f"ps_av{hh}",
                                       bufs=1, name=f"po{hh}_{p}_{qblk}")
                               for hh in range(2)]
                        for kc in range(nkc):
                            d = kc - 4 * qblk
                            c0 = min(KC * max(d, 0), 2 * KC)
                            for hh in range(2):
                                nc.tensor.matmul(
                                    pos[hh][:, c0:SB],
                                    V[2 * p + hh][kc // 4][:,
                                        (kc % 4) * (DH + 1):
                                        (kc % 4 + 1) * (DH + 1)],
                                    P[kc][:, hh * SB + c0:(hh + 1) * SB],
                                    start=(kc == 0),
                                    stop=(kc == nkc - 1),
                                )
                        for hh in range(2):
                            po = pos[hh]
                            # free the psum bank immediately; normalize later
                            avst = ep.tile([DH + 1, SB], f32, tag="avst", bufs=4)
                            nc.vector.tensor_copy(avst[:], po[:])
                            den0 = ep.tile([1, SB], f32, tag="den0", bufs=1)
                            nc.vector.tensor_copy(den0[:], avst[DH:DH + 1, :])
                            rden = ep.tile([1, SB], f32, tag="rden", bufs=1)
                            rscr = ep.tile([1, SB], f32, tag="rscr", bufs=1)
                            nc.vector.reciprocal_approx_accurate(
                                rden[:], den0[:], rscr[:])
                            rden_bc = ep.tile([DH, SB], f32, tag="rden_bc", bufs=2)
                            nc.gpsimd.partition_broadcast(
                                out_ap=rden_bc[:], in_ap=rden[:]
                            )
                            r0 = hh * DH
                            nc.vector.tensor_mul(
                                O[p][r0:r0 + DH, qblk * SB:(qblk + 1) * SB],
                                avst[0:DH, :],
                                rden_bc[:],
                            )
                        # stage this (batch, quarter) block for the AllToAll
                        nc.sync.dma_start(
                            out=a2a_in[4 * p + qblk],
                            in_=O[p][:, qblk * SB:(qblk + 1) * SB],
                        )

            # ---- phase 3: head exchange + output projection --------------
            nc.gpsimd.collective_compute(
                "AllToAll",
                mybir.AluOpType.bypass,
                replica_groups=[[0, 1, 2, 3, 4, 5, 6, 7]],
                ins=[a2a_in[:]],
                outs=[a2a_out[:]],
            )
            with (
                tc.tile_pool(name="wout_pool", bufs=1) as wop,
                tc.tile_pool(name="recv", bufs=1) as rp,
                tc.tile_pool(name="psum_out", bufs=1, space="PSUM") as pu,
            ):
                wout_t = [wop.tile([KC, D], fr, tag=f"wo{k}", name=f"wo{k}")
                          for k in range(NDC)]
                for k in range(NDC):
                    nc.sync.dma_start(out=wout_t[k][:], in_=wout[k])
                # a2a_out block i = heads (2i, 2i+1) of my batch for my
                # quarter -> flat [1024, 512] = attnout^T in global head order
                recv = [rp.tile([KC, SB], fr, tag=f"rc{k}", name=f"rc{k}")
                        for k in range(NDC)]
                for k in range(NDC):
                    eng = nc.sync if k % 2 == 0 else nc.gpsimd
                    eng.dma_start(out=recv[k][:], in_=a2a_out[k])
                for sc in range(SB // KC):
                    for nb in range(D // SB):
                        ps = pu.tile([KC, SB], f32, tag="ps_o", bufs=4)
                        for k in range(NDC):
                            nc.tensor.matmul(
                                ps[:],
                                recv[k][:, sc * KC:(sc + 1) * KC],
                                wout_t[k][:, nb * SB:(nb + 1) * SB],
                                start=(k == 0),
                                stop=(k == NDC - 1),
                            )
                        ot = ep.tile([KC, SB], f32, tag="osb", bufs=4)
                        nc.vector.tensor_add(
                            ot[:], ps[:], bo_bc[:, nb * SB:(nb + 1) * SB]
                        )
                        nc.sync.dma_start(
                            out=out_ext[sc * KC:(sc + 1) * KC,
                                        nb * SB:(nb + 1) * SB],
                            in_=ot[:],
                        )

    nc.compile()
    return nc


def _get_program():
    global _compiled
    if _compiled is None:
        _compiled = _build()
    return _compiled


def _shard_inputs(x, Wqkv, bqkv, Wout, bout):
    """Build the 8 per-core input maps (all host-side numpy)."""
    x = np.ascontiguousarray(x, dtype=np.float32)
    Wqkv = np.asarray(Wqkv, dtype=np.float32)
    bqkv = np.asarray(bqkv, dtype=np.float32)
    Wout = np.asarray(Wout, dtype=np.float32)
    bout = np.ascontiguousarray(np.asarray(bout, dtype=np.float32))

    Wq = Wqkv[:, 0 * D:1 * D]
    Wk = Wqkv[:, 1 * D:2 * D]
    Wv_full = Wqkv[:, 2 * D:3 * D]
    bq = bqkv[0 * D:1 * D]
    bk = bqkv[1 * D:2 * D]
    bv_full = bqkv[2 * D:3 * D]

    # shared across all cores
    xt = np.ascontiguousarray(
        x.transpose(0, 2, 1)                      # [B, D, S]
         .reshape(B, D, NSB, SB).transpose(0, 2, 1, 3)
         .reshape(B, NSB, NDC, KC, SB)
    )
    wout_b = np.ascontiguousarray(Wout.reshape(NDC, KC, D))
    vones = np.ones((KC, NKC), dtype=np.float32)

    in_maps = []
    for c in range(NCORES):
        ha, hb = 2 * c, 2 * c + 1
        wqk_c = np.ascontiguousarray(np.concatenate(
            [Wq[:, ha * DH:(ha + 1) * DH], Wq[:, hb * DH:(hb + 1) * DH],
             Wk[:, ha * DH:(ha + 1) * DH], Wk[:, hb * DH:(hb + 1) * DH]],
            axis=1).reshape(NDC, KC, 2 * KC))
        bqk_c = np.ascontiguousarray(np.concatenate(
            [bq[ha * DH:(ha + 1) * DH], bq[hb * DH:(hb + 1) * DH],
             bk[ha * DH:(ha + 1) * DH], bk[hb * DH:(hb + 1) * DH]]))
        # Wv zero-padded to 256 columns so the V matmul moving dim is 256
        wv_c = np.zeros((D, 2 * KC), dtype=np.float32)
        wv_c[:, 0:DH] = Wv_full[:, ha * DH:(ha + 1) * DH]
        wv_c[:, DH:2 * DH] = Wv_full[:, hb * DH:(hb + 1) * DH]
        wv_c = np.ascontiguousarray(wv_c.reshape(NDC, KC, 2 * KC))
        bv_c = np.ascontiguousarray(np.concatenate(
            [bv_full[ha * DH:(ha + 1) * DH], bv_full[hb * DH:(hb + 1) * DH]]))
        in_maps.append({
            "xt": xt, "wqk": wqk_c, "wv": wv_c, "wout": wout_b,
            "bqk": bqk_c, "bv": bv_c, "bo": bout, "vones": vones,
        })
    return in_maps


def run(inputs, trace=False, trace_kwargs=None):
    nc = _get_program()
    in_maps = _shard_inputs(**inputs)
    res = run_bass_kernel_spmd(
        nc, in_maps, list(range(NCORES)), trace=trace,
        **(trace_kwargs or {}),
    )
    out = np.empty((B, S, D), dtype=np.float32)
    for c in range(NCORES):
        b = c // 4
        r0 = SB * (c % 4)
        out[b, r0:r0 + SB, :] = res.results[c]["out"]
    return out, res


def kernel(**inputs):
    out, _ = run(inputs)
    return out



# revision 6
# speedup vs baseline: 1.2220x; 1.2220x over previous
"""Causal multi-head attention (B=2, S=2048, D=1024, H=16) on 8 trn2 cores.

Sharding: core c handles heads {2c, 2c+1} of BOTH batches (4 (b,h) pairs).
All matmul inputs are bf16 (host-rounded); accumulation stays fp32 in PSUM.

Per core:
  - project host-pretransposed x_b^T [D, S] (both batches) through the
    core's Wqkv column slice into Q^T/K^T head-pair tiles (bf16) and V
    natural tiles with a fused ones-column (AV emits softmax denominators),
  - causal attention per (batch, head-pair) in transposed layout:
    scores^T = K Q^T chunks as two row-tiled (tile_position) matmuls that
    stream concurrently, exp on ScalarE (bf16 out), diagonal masks via
    affine_select, A^T V accumulation on PE,
  - the head exchange is FOUR quarter-wise 8-way AllToAlls, fired as each
    sequence quarter finishes attention, so they overlap later attention.
    Sub-A2A q block t = (my heads, batch t//4, quarter q, col-slice t%4),
    so core i ends up owning tokens {512q + 128*(i%4)} of batch i//4 with
    ALL heads, and runs the output projection per received 128-token chunk.
Host assembles the 8x4 [128, 1024] shards into (2, 2048, 1024).
"""

import sys

for _p in ("/opt/trn_rl_repo", "/opt/pypackages"):
    if _p not in sys.path:
        sys.path.insert(0, _p)

import numpy as np
import ml_dtypes

import concourse.bass as bass
import concourse.mybir as mybir
import concourse.tile as tile
from concourse import bacc
from concourse.bass_utils import run_bass_kernel_spmd

B = 2
S = 2048
D = 1024
H = 16
DH = 64
NCORES = 8
SB = 512           # q block (matmul moving dim)
KC = 128           # k chunk (contraction tile)
NSB = S // SB      # 4 q-blocks
NKC = S // KC      # 16 k-chunks
NDC = D // KC      # 8 contraction chunks for the projections

_compiled = None


def _build():
    f32 = mybir.dt.float32
    bf16 = mybir.dt.bfloat16
    nc = bacc.Bacc(None, target_bir_lowering=False)

    # host-blocked inputs (bf16): xt[b, s, k] = x_b^T[128k:128k+128, 512s:+512]
    xt = nc.declare_dram_parameter("xt", [B, NSB, NDC, KC, SB], bf16, isOutput=False)
    # wqk cols: Q_ha | Q_hb | K_ha | K_hb (64 each)
    wqk = nc.declare_dram_parameter("wqk", [NDC, KC, 2 * KC], bf16, isOutput=False)
    # wv cols: V_ha | V_hb
    wv = nc.declare_dram_parameter("wv", [NDC, KC, KC], bf16, isOutput=False)
    wout = nc.declare_dram_parameter("wout", [NDC, KC, D], bf16, isOutput=False)
    bqk = nc.declare_dram_parameter("bqk", [2 * KC], f32, isOutput=False)
    bv = nc.declare_dram_parameter("bv", [2 * DH], f32, isOutput=False)
    bo = nc.declare_dram_parameter("bo", [D], f32, isOutput=False)
    vones = nc.declare_dram_parameter("vones", [KC, NKC], bf16, isOutput=False)
    # out[q] = final rows for tokens [512q + 128*(c%4), +128) of batch c//4
    out_ext = nc.declare_dram_parameter("out", [NSB, KC, D], f32, isOutput=True)

    # quarter-wise AllToAll staging: sub-A2A q block t =
    #   (my 128 head rows, batch t//4, quarter q, col-slice 128*(t%4))
    a2a_in = [nc.dram_tensor(f"a2a_in{q}", [NCORES, KC, KC], bf16)
              for q in range(NSB)]
    a2a_out = [nc.dram_tensor(f"a2a_out{q}", [NCORES, KC, KC], bf16)
               for q in range(NSB)]

    with tile.TileContext(nc) as tc:
        with (
            tc.tile_pool(name="qkv", bufs=1) as qkvp,
            tc.tile_pool(name="obuf", bufs=1) as op,
            tc.tile_pool(name="misc", bufs=1) as mp,
            tc.tile_pool(name="pbuf", bufs=1) as pb,
            tc.tile_pool(name="evict", bufs=1) as ep,
            tc.tile_pool(name="wpool", bufs=1) as wp,
            tc.tile_pool(name="xbuf", bufs=10) as xp,
            tc.tile_pool(name="recvp", bufs=1) as rp,
            tc.tile_pool(name="psum", bufs=1, space="PSUM") as pp,
        ):
            # ---- weights + small constants --------------------------------
            wqk_t = wp.tile([KC, NDC * 2 * KC], bf16, tag="wqk")
            nc.sync.dma_start(
                out=wqk_t[:].rearrange("p (k c) -> p k c", k=NDC),
                in_=wqk.rearrange("k p c -> p k c"))
            wv_t = wp.tile([KC, NDC * KC], bf16, tag="wv")
            nc.scalar.dma_start(
                out=wv_t[:].rearrange("p (k c) -> p k c", k=NDC),
                in_=wv.rearrange("k p c -> p k c"))
            wout_t = wp.tile([KC, NDC * D], bf16, tag="wout")
            nc.sync.dma_start(
                out=wout_t[:].rearrange("p (k c) -> p k c", k=NDC),
                in_=wout.rearrange("k p c -> p k c"))

            bqk_t = [mp.tile([KC, 1], f32, tag=f"bqk{m}", name=f"bqk{m}")
                     for m in range(2)]
            for m in range(2):
                nc.scalar.dma_start(
                    out=bqk_t[m][:],
                    in_=bqk[m * KC:(m + 1) * KC].rearrange("(p o) -> p o", o=1),
                )
            bv_row = mp.tile([1, 2 * DH], f32, tag="bv_row")
            nc.scalar.dma_start(out=bv_row[:], in_=bv.rearrange("(o f) -> o f", o=1))
            bv_bc = mp.tile([KC, 2 * DH], f32, tag="bv_bc")
            nc.gpsimd.partition_broadcast(out_ap=bv_bc[:], in_ap=bv_row[:])
            bo_row = mp.tile([1, D], f32, tag="bo_row")
            nc.scalar.dma_start(out=bo_row[:], in_=bo.rearrange("(o f) -> o f", o=1))
            bo_bc = mp.tile([KC, D], f32, tag="bo_bc")
            nc.gpsimd.partition_broadcast(out_ap=bo_bc[:], in_ap=bo_row[:])
            vones_sb = mp.tile([KC, NKC], bf16, tag="vones_sb")
            nc.scalar.dma_start(out=vones_sb[:], in_=vones[:])

            # ---- persistent activations -----------------------------------
            # QQ[p][s]: rows 0:64 = Q^T head 2c, 64:128 = head 2c+1 (batch p)
            QQ = [[qkvp.tile([KC, SB], bf16, tag=f"QQ{p}_{s}", name=f"QQ{p}_{s}")
                   for s in range(NSB)] for p in range(B)]
            KK = [[qkvp.tile([KC, SB], bf16, tag=f"KK{p}_{s}", name=f"KK{p}_{s}")
                   for s in range(NSB)] for p in range(B)]
            # V[2p+hh][s]: [128, 4*65]; chunk sc at cols 65sc..+64, col 65sc+64=1
            NCS = SB // KC
            V = [[qkvp.tile([KC, NCS * (DH + 1)], bf16, tag=f"V{v}_{s}",
                            name=f"V{v}_{s}")
                  for s in range(NSB)] for v in range(2 * B)]
            for v in range(2 * B):
                for s in range(NSB):
                    vv = V[v][s][:].rearrange("p (k c) -> p k c", c=DH + 1)
                    nc.vector.tensor_copy(
                        vv[:, :, DH], vones_sb[:, s * NCS:(s + 1) * NCS])
            # O[p]: rows 0:64 = head 2c out^T (normalized), 64:128 = head 2c+1
            O = [op.tile([KC, S], bf16, tag=f"O{p}", name=f"O{p}")
                 for p in range(B)]

            # P score tiles (bf16, post-exp).  Tag per k-chunk; low chunks
            # are double-buffered since they recur every quarter.
            def p_tile(p, q, kc):
                return pb.tile([KC, 2, SB], bf16, tag=f"P{kc}",
                               name=f"P{p}_{q}_{kc}",
                               bufs=(2 if kc < 8 else 1))

            # ---------------------------------------------------------------
            def proj(sblk):
                for bb in range(B):
                    xs = []
                    for j in range(NDC // 2):
                        xtl = xp.tile([KC, 2 * SB], bf16, tag="xt")
                        eng = (nc.sync, nc.gpsimd, nc.scalar, nc.gpsimd)[j]
                        eng.dma_start(
                            out=xtl[:].rearrange("p (k t) -> p k t", k=2),
                            in_=xt[bb, sblk, 2 * j:2 * j + 2].rearrange(
                                "k p t -> p k t"),
                        )
                        xs.append(xtl)

                    def xchunk(k, lo=0, n=SB):
                        return xs[k // 2][:, (k % 2) * SB + lo:
                                          (k % 2) * SB + lo + n]

                    # m = 0 -> Q^T pair, m = 1 -> K^T pair
                    for m in range(2):
                        ps = pp.tile([KC, SB], f32, tag="ps_qk", bufs=1)
                        for k in range(NDC):
                            nc.tensor.matmul(
                                ps[:],
                                wqk_t[:, (2 * k + m) * KC:(2 * k + m + 1) * KC],
                                xchunk(k),
                                start=(k == 0),
                                stop=(k == NDC - 1),
                            )
                        dest = (QQ if m == 0 else KK)[bb][sblk]
                        nc.vector.tensor_scalar_add(dest[:], ps[:], bqk_t[m][:])
                    # V natural: lhsT = x^T token chunk, rhs = Wv [128, 128]
                    for sc in range(NCS):
                        ps = pp.tile([KC, KC], f32, tag="ps_v", bufs=1)
                        for k in range(NDC):
                            nc.tensor.matmul(
                                ps[:],
                                xchunk(k, sc * KC, KC),
                                wv_t[:, k * KC:(k + 1) * KC],
                                start=(k == 0),
                                stop=(k == NDC - 1),
                            )
                        for hh in range(2):
                            nc.vector.tensor_add(
                                V[2 * bb + hh][sblk][:].rearrange(
                                    "p (k c) -> p k c", c=DH + 1)[:, sc, 0:DH],
                                ps[:, hh * DH:(hh + 1) * DH],
                                bv_bc[:, hh * DH:(hh + 1) * DH],
                            )

            # ---------------------------------------------------------------
            def attention(qblk):
                nkc = 4 * (qblk + 1)
                for p in range(B):
                    pos = [pp.tile([DH + 1, SB], f32, tag=f"ps_av{hh}",
                                   bufs=1, name=f"po{hh}_{p}_{qblk}")
                           for hh in range(2)]
                    P = [None] * nkc

                    def scores(kc):
                        d = kc - 4 * qblk
                        c0 = KC * max(d, 0)
                        ps = pp.tile([KC, 2, SB], f32, tag="ps_s", bufs=2)
                        for hh in range(2):  # row-tiled, stream concurrently
                            r0 = hh * DH
                            nc.tensor.matmul(
                                ps[:, hh, c0:SB],
                                KK[p][kc // 4][r0:r0 + DH,
                                               (kc % 4) * KC:(kc % 4 + 1) * KC],
                                QQ[p][qblk][r0:r0 + DH, c0:SB],
                                start=True,
                                stop=True,
                            )
                        P[kc] = p_tile(p, qblk, kc)
                        nc.scalar.activation(
                            P[kc][:, :, c0:SB],
                            ps[:, :, c0:SB],
                            mybir.ActivationFunctionType.Exp,
                            scale=1.0 / float(np.sqrt(DH)),
                        )
                        if d >= 0:  # diagonal chunk: zero where k > q
                            nc.gpsimd.affine_select(
                                out=P[kc][:, :, c0:SB],
                                in_=P[kc][:, :, c0:SB],
                                pattern=[[0, 2], [1, SB - c0]],
                                compare_op=mybir.AluOpType.is_ge,
                                fill=0.0,
                                base=0,
                                channel_multiplier=-1,
                            )

                    def av(kc):
                        d = kc - 4 * qblk
                        c0 = KC * max(d, 0)
                        for hh in range(2):
                            nc.tensor.matmul(
                                pos[hh][:, c0:SB],
                                V[2 * p + hh][kc // 4][:,
                                    (kc % 4) * (DH + 1):
                                    (kc % 4 + 1) * (DH + 1)],
                                P[kc][:, hh, c0:SB],
                                start=(kc == 0),
                                stop=(kc == nkc - 1),
                            )

                    # interleave: sc(kc) | av(kc-1) keeps ScalarE saturated
                    for kc in range(nkc):
                        scores(kc)
                        if kc >= 1:
                            av(kc - 1)
                    av(nkc - 1)

                    for hh in range(2):
                        po = pos[hh]
                        avst = ep.tile([DH + 1, SB], f32, tag="avst", bufs=4)
                        nc.vector.tensor_copy(avst[:], po[:])
                        den0 = ep.tile([1, SB], f32, tag="den0", bufs=1)
                        nc.vector.tensor_copy(den0[:], avst[DH:DH + 1, :])
                        rden = ep.tile([1, SB], f32, tag="rden", bufs=1)
                        rscr = ep.tile([1, SB], f32, tag="rscr", bufs=1)
                        nc.vector.reciprocal_approx_accurate(
                            rden[:], den0[:], rscr[:])
                        rden_bc = ep.tile([DH, SB], f32, tag="rden_bc", bufs=2)
                        nc.gpsimd.partition_broadcast(
                            out_ap=rden_bc[:], in_ap=rden[:])
                        r0 = hh * DH
                        nc.vector.tensor_mul(
                            O[p][r0:r0 + DH, qblk * SB:(qblk + 1) * SB],
                            avst[0:DH, :],
                            rden_bc[:],
                        )
                    # stage this (batch, quarter) into the sub-A2A buffer
                    nc.gpsimd.dma_start(
                        out=a2a_in[qblk][4 * p:4 * p + 4].rearrange(
                            "t p c -> p t c"),
                        in_=O[p][:, qblk * SB:(qblk + 1) * SB].rearrange(
                            "p (t c) -> p t c", t=4),
                    )
                nc.gpsimd.collective_compute(
                    "AllToAll",
                    mybir.AluOpType.bypass,
                    replica_groups=[[0, 1, 2, 3, 4, 5, 6, 7]],
                    ins=[a2a_in[qblk][:]],
                    outs=[a2a_out[qblk][:]],
                )

            # ---------------------------------------------------------------
            def outproj(qblk):
                recv = []
                for m in range(NDC // 2):
                    rt = rp.tile([KC, 2 * KC], bf16, tag=f"rc{m}",
                                 name=f"rc{m}_{qblk}")
                    nc.sync.dma_start(
                        out=rt[:].rearrange("p (t c) -> p t c", t=2),
                        in_=a2a_out[qblk][2 * m:2 * m + 2].rearrange(
                            "t p c -> p t c"),
                    )
                    recv.append(rt)
                for nb in range(D // SB):
                    ps = pp.tile([KC, SB], f32, tag="ps_qk", bufs=1)
                    for k in range(NDC):
                        nc.tensor.matmul(
                            ps[:],
                            recv[k // 2][:, (k % 2) * KC:(k % 2 + 1) * KC],
                            wout_t[:, k * D + nb * SB:k * D + (nb + 1) * SB],
                            start=(k == 0),
                            stop=(k == NDC - 1),
                        )
                    ot = ep.tile([KC, SB], f32, tag="osb", bufs=4)
                    nc.vector.tensor_add(
                        ot[:], ps[:], bo_bc[:, nb * SB:(nb + 1) * SB])
                    nc.sync.dma_start(
                        out=out_ext[qblk][:, nb * SB:(nb + 1) * SB],
                        in_=ot[:],
                    )

            # ---- static schedule ------------------------------------------
            proj(0)
            attention(0)
            proj(1)
            attention(1)
            proj(2)
            outproj(0)
            attention(2)
            proj(3)
            outproj(1)
            attention(3)
            outproj(2)
            outproj(3)

    nc.compile()
    return nc


def _get_program():
    global _compiled
    if _compiled is None:
        _compiled = _build()
    return _compiled


def _shard_inputs(x, Wqkv, bqkv, Wout, bout):
    """Build the 8 per-core input maps (all host-side numpy, bf16 data)."""
    bf = ml_dtypes.bfloat16
    x = np.asarray(x, dtype=np.float32)
    Wqkv = np.asarray(Wqkv, dtype=np.float32)
    bqkv = np.ascontiguousarray(np.asarray(bqkv, dtype=np.float32))
    Wout = np.asarray(Wout, dtype=np.float32)
    bout = np.ascontiguousarray(np.asarray(bout, dtype=np.float32))

    Wq = Wqkv[:, 0 * D:1 * D]
    Wk = Wqkv[:, 1 * D:2 * D]
    Wv_full = Wqkv[:, 2 * D:3 * D]
    bq = bqkv[0 * D:1 * D]
    bk = bqkv[1 * D:2 * D]
    bv_full = bqkv[2 * D:3 * D]

    # shared across all cores
    xt = np.ascontiguousarray(
        x.transpose(0, 2, 1)                      # [B, D, S]
         .reshape(B, D, NSB, SB).transpose(0, 2, 1, 3)
         .reshape(B, NSB, NDC, KC, SB).astype(bf)
    )
    wout_b = np.ascontiguousarray(Wout.reshape(NDC, KC, D).astype(bf))
    vones = np.ones((KC, NKC), dtype=bf)

    in_maps = []
    for c in range(NCORES):
        ha, hb = 2 * c, 2 * c + 1
        # wqk col layout interleaved by k-chunk is handled by the kernel's
        # single [128, NDC*256] sbuf tile: chunk k at cols k*256..(k+1)*256
        # with [Q_ha Q_hb](128) then [K_ha K_hb](128)
        wqk_c = np.ascontiguousarray(np.concatenate(
            [Wq[:, ha * DH:(ha + 1) * DH], Wq[:, hb * DH:(hb + 1) * DH],
             Wk[:, ha * DH:(ha + 1) * DH], Wk[:, hb * DH:(hb + 1) * DH]],
            axis=1).reshape(NDC, KC, 2 * KC).astype(bf))
        bqk_c = np.ascontiguousarray(np.concatenate(
            [bq[ha * DH:(ha + 1) * DH], bq[hb * DH:(hb + 1) * DH],
             bk[ha * DH:(ha + 1) * DH], bk[hb * DH:(hb + 1) * DH]]))
        wv_c = np.ascontiguousarray(np.concatenate(
            [Wv_full[:, ha * DH:(ha + 1) * DH],
             Wv_full[:, hb * DH:(hb + 1) * DH]],
            axis=1).reshape(NDC, KC, KC).astype(bf))
        bv_c = np.ascontiguousarray(np.concatenate(
            [bv_full[ha * DH:(ha + 1) * DH], bv_full[hb * DH:(hb + 1) * DH]]))
        in_maps.append({
            "xt": xt, "wqk": wqk_c, "wv": wv_c, "wout": wout_b,
            "bqk": bqk_c, "bv": bv_c, "bo": bout, "vones": vones,
        })
    return in_maps


def run(inputs, trace=False, trace_kwargs=None):
    nc = _get_program()
    in_maps = _shard_inputs(**inputs)
    res = run_bass_kernel_spmd(
        nc, in_maps, list(range(NCORES)), trace=trace,
        **(trace_kwargs or {}),
    )
    out = np.empty((B, S, D), dtype=np.float32)
    for c in range(NCORES):
        b = c // 4
        t4 = c % 4
        oc = res.results[c]["out"]  # [NSB, KC, D]
        for q in range(NSB):
            out[b, SB * q + KC * t4: SB * q + KC * (t4 + 1), :] = oc[q]
    return out, res


def kernel(**inputs):
    out, _ = run(inputs)
    return out


# revision 9
# speedup vs baseline: 1.2731x; 1.0419x over previous
"""Causal multi-head attention (B=2, S=2048, D=1024, H=16) on 8 trn2 cores.

Sharding: core c handles heads {2c, 2c+1} of BOTH batches (4 (b,h) pairs).
All matmul inputs are bf16 (host-rounded); accumulation stays fp32 in PSUM.

Per core:
  - project host-pretransposed x_b^T [D, S] (both batches) through the
    core's Wqkv column slice into Q^T/K^T head-pair tiles (bf16).  V is
    produced transposed (long moving dim), then flipped to natural layout
    with PE transposes; a fused ones-column makes AV emit softmax
    denominators,
  - causal attention per (batch, head-pair) in transposed layout:
    scores^T = K Q^T chunks as two row-tiled (tile_position) matmuls that
    stream concurrently, exp on ScalarE (bf16 out), diagonal masks via
    affine_select, A^T V accumulation on PE,
  - the head exchange is FOUR quarter-wise 8-way AllToAlls, fired as each
    sequence quarter finishes attention, so they overlap later attention.
    Sub-A2A q block t = (my heads, batch t//4, quarter q, col-slice t%4),
    so core i ends up owning tokens {512q + 128*(i%4)} of batch i//4 with
    ALL heads, and runs the output projection per received 128-token chunk.
    A dummy AllToAll issued at program start absorbs the ~11us collective
    firmware warmup.
Host assembles the 8x4 [128, 1024] shards into (2, 2048, 1024).
"""

import sys

for _p in ("/opt/trn_rl_repo", "/opt/pypackages"):
    if _p not in sys.path:
        sys.path.insert(0, _p)

import numpy as np
import ml_dtypes

import concourse.bass as bass
import concourse.mybir as mybir
import concourse.tile as tile
from concourse import bacc
from concourse.bass_utils import run_bass_kernel_spmd

B = 2
S = 2048
D = 1024
H = 16
DH = 64
NCORES = 8
SB = 512           # q block (matmul moving dim)
KC = 128           # k chunk (contraction tile)
NSB = S // SB      # 4 q-blocks
NKC = S // KC      # 16 k-chunks
NDC = D // KC      # 8 contraction chunks for the projections

_compiled = None


def _build():
    f32 = mybir.dt.float32
    bf16 = mybir.dt.bfloat16
    nc = bacc.Bacc(None, target_bir_lowering=False)

    # host-blocked inputs (bf16): xt[b, s, k] = x_b^T[128k:128k+128, 512s:+512]
    xt = nc.declare_dram_parameter("xt", [B, NSB, NDC, KC, SB], bf16, isOutput=False)
    # wqk cols: Q_ha | Q_hb | K_ha | K_hb (64 each)
    wqk = nc.declare_dram_parameter("wqk", [NDC, KC, 2 * KC], bf16, isOutput=False)
    # wv cols: V_ha | V_hb
    wv = nc.declare_dram_parameter("wv", [NDC, KC, KC], bf16, isOutput=False)
    wout = nc.declare_dram_parameter("wout", [NDC, KC, D], bf16, isOutput=False)
    bqk = nc.declare_dram_parameter("bqk", [2 * KC], f32, isOutput=False)
    bv = nc.declare_dram_parameter("bv", [2 * DH], f32, isOutput=False)
    bo = nc.declare_dram_parameter("bo", [D], f32, isOutput=False)
    vones = nc.declare_dram_parameter("vones", [KC, NKC], bf16, isOutput=False)
    ident = nc.declare_dram_parameter("ident", [KC, KC], bf16, isOutput=False)
    # out[q] = final rows for tokens [512q + 128*(c%4), +128) of batch c//4
    out_ext = nc.declare_dram_parameter("out", [NSB, KC, D], f32, isOutput=True)

    # quarter-wise AllToAll staging: sub-A2A q block t =
    #   (my 128 head rows, batch t//4, quarter q, col-slice 128*(t%4))
    a2a_in = [nc.dram_tensor(f"a2a_in{q}", [NCORES, KC, KC], bf16)
              for q in range(NSB)]
    a2a_out = [nc.dram_tensor(f"a2a_out{q}", [NCORES, KC, KC], bf16)
               for q in range(NSB)]
    ccw_in = nc.dram_tensor("ccw_in", [NCORES, 1, 2], bf16)
    ccw_out = nc.dram_tensor("ccw_out", [NCORES, 1, 2], bf16)

    with tile.TileContext(nc) as tc:
        with (
            tc.tile_pool(name="qkv", bufs=1) as qkvp,
            tc.tile_pool(name="obuf", bufs=1) as op,
            tc.tile_pool(name="misc", bufs=1) as mp,
            tc.tile_pool(name="pbuf", bufs=1) as pb,
            tc.tile_pool(name="evict", bufs=1) as ep,
            tc.tile_pool(name="wpool", bufs=1) as wp,
            tc.tile_pool(name="xbuf", bufs=10) as xp,
            tc.tile_pool(name="recvp", bufs=1) as rp,
            tc.tile_pool(name="psum", bufs=1, space="PSUM") as pp,
        ):
            # ---- collective firmware warmup (absorbs ~11us trigger delay)
            nc.gpsimd.collective_compute(
                "AllToAll",
                mybir.AluOpType.bypass,
                replica_groups=[[0, 1, 2, 3, 4, 5, 6, 7]],
                ins=[ccw_in[:]],
                outs=[ccw_out[:]],
            )

            # ---- weights + small constants --------------------------------
            # sync queue: wqk first (needed by the very first matmul), then
            # the first x tiles.  Big wout load rides the vector queue.
            wqk_t = wp.tile([KC, NDC * 2 * KC], bf16, tag="wqk")
            nc.sync.dma_start(
                out=wqk_t[:].rearrange("p (k c) -> p k c", k=NDC),
                in_=wqk.rearrange("k p c -> p k c"))
            bqk_t = [mp.tile([KC, 1], f32, tag=f"bqk{m}", name=f"bqk{m}")
                     for m in range(2)]
            for m in range(2):
                nc.scalar.dma_start(
                    out=bqk_t[m][:],
                    in_=bqk[m * KC:(m + 1) * KC].rearrange("(p o) -> p o", o=1),
                )
            bv_t = mp.tile([KC, 1], f32, tag="bv_t")
            nc.scalar.dma_start(
                out=bv_t[:], in_=bv.rearrange("(p o) -> p o", o=1))
            ident_t = mp.tile([KC, KC], bf16, tag="ident")
            nc.scalar.dma_start(out=ident_t[:], in_=ident[:])
            vones_sb = mp.tile([KC, NKC], bf16, tag="vones_sb")
            nc.scalar.dma_start(out=vones_sb[:], in_=vones[:])
            wv_t = wp.tile([KC, NDC * KC], bf16, tag="wv")
            nc.scalar.dma_start(
                out=wv_t[:].rearrange("p (k c) -> p k c", k=NDC),
                in_=wv.rearrange("k p c -> p k c"))
            wout_t = wp.tile([KC, NDC * D], bf16, tag="wout")
            nc.scalar.dma_start(
                out=wout_t[:].rearrange("p (k c) -> p k c", k=NDC),
                in_=wout.rearrange("k p c -> p k c"))
            bo_row = mp.tile([1, D], f32, tag="bo_row")
            nc.scalar.dma_start(out=bo_row[:], in_=bo.rearrange("(o f) -> o f", o=1))
            bo_bc = mp.tile([KC, D], f32, tag="bo_bc")
            nc.gpsimd.partition_broadcast(out_ap=bo_bc[:], in_ap=bo_row[:])

            # ---- persistent activations -----------------------------------
            # QQ[p][s]: rows 0:64 = Q^T head 2c, 64:128 = head 2c+1 (batch p)
            QQ = [[qkvp.tile([KC, SB], bf16, tag=f"QQ{p}_{s}", name=f"QQ{p}_{s}")
                   for s in range(NSB)] for p in range(B)]
            KK = [[qkvp.tile([KC, SB], bf16, tag=f"KK{p}_{s}", name=f"KK{p}_{s}")
                   for s in range(NSB)] for p in range(B)]
            # V[2p+hh][s]: [128, 4*65]; chunk sc at cols 65sc..+64, col 65sc+64=1
            NCS = SB // KC
            V = [[qkvp.tile([KC, NCS * (DH + 1)], bf16, tag=f"V{v}_{s}",
                            name=f"V{v}_{s}")
                  for s in range(NSB)] for v in range(2 * B)]
            for v in range(2 * B):
                for s in range(NSB):
                    vv = V[v][s][:].rearrange("p (k c) -> p k c", c=DH + 1)
                    nc.vector.tensor_copy(
                        vv[:, :, DH], vones_sb[:, s * NCS:(s + 1) * NCS])
            # O[p]: rows 0:64 = head 2c out^T (normalized), 64:128 = head 2c+1
            O = [op.tile([KC, S], bf16, tag=f"O{p}", name=f"O{p}")
                 for p in range(B)]

            # P score tiles (bf16, post-exp).  Tag per k-chunk; low chunks
            # are double-buffered since they recur every quarter.
            def p_tile(p, q, kc):
                return pb.tile([KC, 2, SB], bf16, tag=f"P{kc}",
                               name=f"P{p}_{q}_{kc}",
                               bufs=(2 if kc < 8 else 1))

            # ---------------------------------------------------------------
            def proj(sblk):
                vts = []
                for bb in range(B):
                    xs = []
                    for j in range(NDC // 2):
                        xtl = xp.tile([KC, 2 * SB], bf16, tag="xt")
                        eng = (nc.sync, nc.gpsimd, nc.scalar, nc.gpsimd)[j]
                        eng.dma_start(
                            out=xtl[:].rearrange("p (k t) -> p k t", k=2),
                            in_=xt[bb, sblk, 2 * j:2 * j + 2].rearrange(
                                "k p t -> p k t"),
                        )
                        xs.append(xtl)

                    def xchunk(k):
                        return xs[k // 2][:, (k % 2) * SB:(k % 2 + 1) * SB]

                    # m = 0 -> Q^T pair, m = 1 -> K^T pair
                    for m in range(2):
                        ps = pp.tile([KC, SB], f32, tag="ps_qk", bufs=1)
                        for k in range(NDC):
                            nc.tensor.matmul(
                                ps[:],
                                wqk_t[:, (2 * k + m) * KC:(2 * k + m + 1) * KC],
                                xchunk(k),
                                start=(k == 0),
                                stop=(k == NDC - 1),
                            )
                        dest = (QQ if m == 0 else KK)[bb][sblk]
                        nc.vector.tensor_scalar_add(dest[:], ps[:], bqk_t[m][:])
                    # V^T: long moving dim, then flip via PE transposes below
                    ps = pp.tile([KC, SB], f32, tag="ps_qk", bufs=1)
                    for k in range(NDC):
                        nc.tensor.matmul(
                            ps[:],
                            wv_t[:, k * KC:(k + 1) * KC],
                            xchunk(k),
                            start=(k == 0),
                            stop=(k == NDC - 1),
                        )
                    vt = ep.tile([KC, SB], bf16, tag="vt", bufs=2,
                                 name=f"vt{bb}_{sblk}")
                    nc.vector.tensor_scalar_add(vt[:], ps[:], bv_t[:])
                    vts.append(vt)
                for bb in range(B):
                    pst = pp.tile([KC, SB], bf16, tag="ps_tr", bufs=1)
                    for sc in range(NCS):
                        nc.tensor.transpose(
                            pst[:, sc * KC:(sc + 1) * KC],
                            vts[bb][:, sc * KC:(sc + 1) * KC],
                            ident_t[:],
                        )
                    ps4 = pst[:].rearrange("p (k h c) -> p k h c", k=NCS, h=2)
                    for hh in range(2):
                        nc.vector.tensor_copy(
                            V[2 * bb + hh][sblk][:].rearrange(
                                "p (k c) -> p k c", c=DH + 1)[:, :, 0:DH],
                            ps4[:, :, hh, :],
                        )

            # ---------------------------------------------------------------
            def attention(qblk):
                nkc = 4 * (qblk + 1)
                for p in range(B):
                    pos = [pp.tile([DH + 1, SB], f32, tag=f"ps_av{hh}",
                                   bufs=1, name=f"po{hh}_{p}_{qblk}")
                           for hh in range(2)]
                    P = [None] * nkc

                    def scores(kc):
                        d = kc - 4 * qblk
                        c0 = KC * max(d, 0)
                        ps = pp.tile([KC, 2, SB], f32, tag="ps_s", bufs=2)
                        for hh in range(2):  # row-tiled, stream concurrently
                            r0 = hh * DH
                            nc.tensor.matmul(
                                ps[:, hh, c0:SB],
                                KK[p][kc // 4][r0:r0 + DH,
                                               (kc % 4) * KC:(kc % 4 + 1) * KC],
                                QQ[p][qblk][r0:r0 + DH, c0:SB],
                                start=True,
                                stop=True,
                            )
                        P[kc] = p_tile(p, qblk, kc)
                        nc.scalar.activation(
                            P[kc][:, :, c0:SB],
                            ps[:, :, c0:SB],
                            mybir.ActivationFunctionType.Exp,
                            scale=1.0 / float(np.sqrt(DH)),
                        )
                        if d >= 0:  # diagonal chunk: zero where k > q
                            nc.gpsimd.affine_select(
                                out=P[kc][:, :, c0:SB],
                                in_=P[kc][:, :, c0:SB],
                                pattern=[[0, 2], [1, SB - c0]],
                                compare_op=mybir.AluOpType.is_ge,
                                fill=0.0,
                                base=0,
                                channel_multiplier=-1,
                            )

                    def av(kc):
                        d = kc - 4 * qblk
                        c0 = KC * max(d, 0)
                        for hh in range(2):
                            nc.tensor.matmul(
                                pos[hh][:, c0:SB],
                                V[2 * p + hh][kc // 4][:,
                                    (kc % 4) * (DH + 1):
                                    (kc % 4 + 1) * (DH + 1)],
                                P[kc][:, hh, c0:SB],
                                start=(kc == 0),
                                stop=(kc == nkc - 1),
                            )

                    # interleave: sc(kc) | av(kc-1) keeps ScalarE saturated
                    for kc in range(nkc):
                        scores(kc)
                        if kc >= 1:
                            av(kc - 1)
                    av(nkc - 1)

                    # normalize (PSUM reads stay on DVE; broadcast on GpSimd)
                    for hh in range(2):
                        po = pos[hh]
                        den0 = ep.tile([1, SB], f32, tag=f"den{hh}", bufs=1)
                        nc.vector.tensor_copy(den0[:], po[DH:DH + 1, :])
                        rden = ep.tile([1, SB], f32, tag=f"rden{hh}", bufs=1)
                        rscr = ep.tile([1, SB], f32, tag=f"rscr{hh}", bufs=1)
                        nc.vector.reciprocal_approx_accurate(
                            rden[:], den0[:], rscr[:])
                        rden_bc = ep.tile([DH, SB], f32, tag=f"rbc{hh}", bufs=2)
                        nc.gpsimd.partition_broadcast(
                            out_ap=rden_bc[:], in_ap=rden[:])
                        r0 = hh * DH
                        nc.vector.tensor_mul(
                            O[p][r0:r0 + DH, qblk * SB:(qblk + 1) * SB],
                            po[0:DH, :],
                            rden_bc[:],
                        )
                    # stage this (batch, quarter) into the sub-A2A buffer
                    nc.gpsimd.dma_start(
                        out=a2a_in[qblk][4 * p:4 * p + 4].rearrange(
                            "t p c -> p t c"),
                        in_=O[p][:, qblk * SB:(qblk + 1) * SB].rearrange(
                            "p (t c) -> p t c", t=4),
                    )
                nc.gpsimd.collective_compute(
                    "AllToAll",
                    mybir.AluOpType.bypass,
                    replica_groups=[[0, 1, 2, 3, 4, 5, 6, 7]],
                    ins=[a2a_in[qblk][:]],
                    outs=[a2a_out[qblk][:]],
                )

            # ---------------------------------------------------------------
            def outproj(qblk):
                recv = []
                for m in range(NDC // 2):
                    rt = rp.tile([KC, 2 * KC], bf16, tag=f"rc{m}",
                                 name=f"rc{m}_{qblk}")
                    nc.sync.dma_start(
                        out=rt[:].rearrange("p (t c) -> p t c", t=2),
                        in_=a2a_out[qblk][2 * m:2 * m + 2].rearrange(
                            "t p c -> p t c"),
                    )
                    recv.append(rt)
                for nb in range(D // SB):
                    ps = pp.tile([KC, SB], f32, tag="ps_qk", bufs=1)
                    for k in range(NDC):
                        nc.tensor.matmul(
                            ps[:],
                            recv[k // 2][:, (k % 2) * KC:(k % 2 + 1) * KC],
                            wout_t[:, k * D + nb * SB:k * D + (nb + 1) * SB],
                            start=(k == 0),
                            stop=(k == NDC - 1),
                        )
                    ot = ep.tile([KC, SB], f32, tag="osb", bufs=4)
                    nc.vector.tensor_add(
                        ot[:], ps[:], bo_bc[:, nb * SB:(nb + 1) * SB])
                    nc.sync.dma_start(
                        out=out_ext[qblk][:, nb * SB:(nb + 1) * SB],
                        in_=ot[:],
                    )

            # ---- static schedule ------------------------------------------
            proj(0)
            attention(0)
            proj(1)
            attention(1)
            proj(2)
            attention(2)
            proj(3)
            outproj(0)
            outproj(1)
            attention(3)
            outproj(2)
            outproj(3)

    nc.compile()
    return nc


def _get_program():
    global _compiled
    if _compiled is None:
        _compiled = _build()
    return _compiled


def _shard_inputs(x, Wqkv, bqkv, Wout, bout):
    """Build the 8 per-core input maps (all host-side numpy, bf16 data)."""
    bf = ml_dtypes.bfloat16
    x = np.asarray(x, dtype=np.float32)
    Wqkv = np.asarray(Wqkv, dtype=np.float32)
    bqkv = np.ascontiguousarray(np.asarray(bqkv, dtype=np.float32))
    Wout = np.asarray(Wout, dtype=np.float32)
    bout = np.ascontiguousarray(np.asarray(bout, dtype=np.float32))

    Wq = Wqkv[:, 0 * D:1 * D]
    Wk = Wqkv[:, 1 * D:2 * D]
    Wv_full = Wqkv[:, 2 * D:3 * D]
    bq = bqkv[0 * D:1 * D]
    bk = bqkv[1 * D:2 * D]
    bv_full = bqkv[2 * D:3 * D]

    # shared across all cores
    xt = np.ascontiguousarray(
        x.transpose(0, 2, 1)                      # [B, D, S]
         .reshape(B, D, NSB, SB).transpose(0, 2, 1, 3)
         .reshape(B, NSB, NDC, KC, SB).astype(bf)
    )
    wout_b = np.ascontiguousarray(Wout.reshape(NDC, KC, D).astype(bf))
    vones = np.ones((KC, NKC), dtype=bf)
    ident = np.eye(KC, dtype=bf)

    in_maps = []
    for c in range(NCORES):
        ha, hb = 2 * c, 2 * c + 1
        wqk_c = np.ascontiguousarray(np.concatenate(
            [Wq[:, ha * DH:(ha + 1) * DH], Wq[:, hb * DH:(hb + 1) * DH],
             Wk[:, ha * DH:(ha + 1) * DH], Wk[:, hb * DH:(hb + 1) * DH]],
            axis=1).reshape(NDC, KC, 2 * KC).astype(bf))
        bqk_c = np.ascontiguousarray(np.concatenate(
            [bq[ha * DH:(ha + 1) * DH], bq[hb * DH:(hb + 1) * DH],
             bk[ha * DH:(ha + 1) * DH], bk[hb * DH:(hb + 1) * DH]]))
        wv_c = np.ascontiguousarray(np.concatenate(
            [Wv_full[:, ha * DH:(ha + 1) * DH],
             Wv_full[:, hb * DH:(hb + 1) * DH]],
            axis=1).reshape(NDC, KC, KC).astype(bf))
        bv_c = np.ascontiguousarray(np.concatenate(
            [bv_full[ha * DH:(ha + 1) * DH], bv_full[hb * DH:(hb + 1) * DH]]))
        in_maps.append({
            "xt": xt, "wqk": wqk_c, "wv": wv_c, "wout": wout_b,
            "bqk": bqk_c, "bv": bv_c, "bo": bout, "vones": vones,
            "ident": ident,
        })
    return in_maps


def run(inputs, trace=False, trace_kwargs=None):
    nc = _get_program()
    in_maps = _shard_inputs(**inputs)
    res = run_bass_kernel_spmd(
        nc, in_maps, list(range(NCORES)), trace=trace,
        **(trace_kwargs or {}),
    )
    out = np.empty((B, S, D), dtype=np.float32)
    for c in range(NCORES):
        b = c // 4
        t4 = c % 4
        oc = res.results[c]["out"]  # [NSB, KC, D]
        for q in range(NSB):
            out[b, SB * q + KC * t4: SB * q + KC * (t4 + 1), :] = oc[q]
    return out, res


def kernel(**inputs):
    out, _ = run(inputs)
    return out


# revision 11
# speedup vs baseline: 1.3721x; 1.0778x over previous
"""Causal multi-head attention (B=2, S=2048, D=1024, H=16) on 8 trn2 cores.

Sharding: core c handles heads {2c, 2c+1} of BOTH batches (4 (b,h) pairs).
All matmul inputs are bf16 (host-rounded); accumulation stays fp32 in PSUM.

Per core:
  - project host-pretransposed x_b^T [D, S] (both batches) through the
    core's Wqkv column slice into Q^T/K^T head-pair tiles (bf16).  V is
    produced transposed (long moving dim), then flipped to natural layout
    with PE transposes; a fused ones-column makes AV emit softmax
    denominators,
  - causal attention per (batch, head-pair) in transposed layout:
    scores^T = K Q^T chunks as two row-tiled (tile_position) matmuls that
    stream concurrently, exp on ScalarE (bf16 out), diagonal masks via
    affine_select, A^T V accumulation on PE,
  - the head exchange is FOUR quarter-wise 8-way AllToAlls, fired as each
    sequence quarter finishes attention, so they overlap later attention.
    Sub-A2A q block t = (my heads, batch t//4, quarter q, col-slice t%4),
    so core i ends up owning tokens {512q + 128*(i%4)} of batch i//4 with
    ALL heads, and runs the output projection per received 128-token chunk.
    A dummy AllToAll issued at program start absorbs the ~11us collective
    firmware warmup.
Host assembles the 8x4 [128, 1024] shards into (2, 2048, 1024).
"""

import sys

for _p in ("/opt/trn_rl_repo", "/opt/pypackages"):
    if _p not in sys.path:
        sys.path.insert(0, _p)

import numpy as np
import ml_dtypes

import concourse.bass as bass
import concourse.mybir as mybir
import concourse.tile as tile
from concourse import bacc
from concourse.bass_utils import run_bass_kernel_spmd

B = 2
S = 2048
D = 1024
H = 16
DH = 64
NCORES = 8
SB = 512           # q block (matmul moving dim)
KC = 128           # k chunk (contraction tile)
NSB = S // SB      # 4 q-blocks
NKC = S // KC      # 16 k-chunks
NDC = D // KC      # 8 contraction chunks for the projections

_compiled = None


def _build():
    f32 = mybir.dt.float32
    bf16 = mybir.dt.bfloat16
    nc = bacc.Bacc(None, target_bir_lowering=False)

    # host-blocked inputs (bf16): xt[b, s, k] = x_b^T[128k:128k+128, 512s:+512]
    xt = nc.declare_dram_parameter("xt", [B, NSB, NDC, KC, SB], bf16, isOutput=False)
    # wqk cols: Q_ha | Q_hb | K_ha | K_hb (64 each)
    wqk = nc.declare_dram_parameter("wqk", [NDC, KC, 2 * KC], bf16, isOutput=False)
    # wv cols: V_ha | V_hb
    wv = nc.declare_dram_parameter("wv", [NDC, KC, KC], bf16, isOutput=False)
    wout = nc.declare_dram_parameter("wout", [NDC, KC, D], bf16, isOutput=False)
    bqk = nc.declare_dram_parameter("bqk", [2 * KC], f32, isOutput=False)
    bv = nc.declare_dram_parameter("bv", [2 * DH], f32, isOutput=False)
    bo = nc.declare_dram_parameter("bo", [D], f32, isOutput=False)
    vones = nc.declare_dram_parameter("vones", [KC, NKC], bf16, isOutput=False)
    ident = nc.declare_dram_parameter("ident", [KC, KC], bf16, isOutput=False)
    # out[q] = final rows for tokens [512q + 128*(c%4), +128) of batch c//4
    out_ext = nc.declare_dram_parameter("out", [NSB, KC, D], f32, isOutput=True)

    # quarter-wise AllToAll staging: sub-A2A q block t =
    #   (my 128 head rows, batch t//4, quarter q, col-slice 128*(t%4))
    a2a_in = [nc.dram_tensor(f"a2a_in{q}", [NCORES, KC, KC], bf16)
              for q in range(NSB)]
    a2a_out = [nc.dram_tensor(f"a2a_out{q}", [NCORES, KC, KC], bf16)
               for q in range(NSB)]
    ccw_in = nc.dram_tensor("ccw_in", [NCORES, 1, 2], bf16)
    ccw_out = nc.dram_tensor("ccw_out", [NCORES, 1, 2], bf16)

    with tile.TileContext(nc) as tc:
        with (
            tc.tile_pool(name="qkv", bufs=1) as qkvp,
            tc.tile_pool(name="obuf", bufs=1) as op,
            tc.tile_pool(name="misc", bufs=1) as mp,
            tc.tile_pool(name="pbuf", bufs=1) as pb,
            tc.tile_pool(name="evict", bufs=1) as ep,
            tc.tile_pool(name="wpool", bufs=1) as wp,
            tc.tile_pool(name="xbuf", bufs=10) as xp,
            tc.tile_pool(name="recvp", bufs=1) as rp,
            tc.tile_pool(name="psum", bufs=1, space="PSUM") as pp,
        ):
            # ---- collective firmware warmup (absorbs ~11us trigger delay)
            nc.gpsimd.collective_compute(
                "AllToAll",
                mybir.AluOpType.bypass,
                replica_groups=[[0, 1, 2, 3, 4, 5, 6, 7]],
                ins=[ccw_in[:]],
                outs=[ccw_out[:]],
            )

            # ---- weights + small constants --------------------------------
            # sync queue: wqk first (needed by the very first matmul), then
            # the first x tiles.  Big wout load rides the vector queue.
            wqk_t = wp.tile([KC, NDC * 2 * KC], bf16, tag="wqk")
            nc.sync.dma_start(
                out=wqk_t[:].rearrange("p (k c) -> p k c", k=NDC),
                in_=wqk.rearrange("k p c -> p k c"))
            bqk_t = [mp.tile([KC, 1], f32, tag=f"bqk{m}", name=f"bqk{m}")
                     for m in range(2)]
            for m in range(2):
                nc.scalar.dma_start(
                    out=bqk_t[m][:],
                    in_=bqk[m * KC:(m + 1) * KC].rearrange("(p o) -> p o", o=1),
                )
            bv_t = mp.tile([KC, 1], f32, tag="bv_t")
            nc.scalar.dma_start(
                out=bv_t[:], in_=bv.rearrange("(p o) -> p o", o=1))
            ident_t = mp.tile([KC, KC], bf16, tag="ident")
            nc.scalar.dma_start(out=ident_t[:], in_=ident[:])
            vones_sb = mp.tile([KC, NKC], bf16, tag="vones_sb")
            nc.scalar.dma_start(out=vones_sb[:], in_=vones[:])
            act_warm = mp.tile([1, 2], f32, tag="act_warm")
            nc.scalar.activation(
                act_warm[:], vones_sb[0:1, 0:2],
                mybir.ActivationFunctionType.Exp, scale=1.0)
            wv_t = wp.tile([KC, NDC * KC], bf16, tag="wv")
            nc.scalar.dma_start(
                out=wv_t[:].rearrange("p (k c) -> p k c", k=NDC),
                in_=wv.rearrange("k p c -> p k c"))
            wout_t = wp.tile([KC, NDC * D], bf16, tag="wout")
            nc.scalar.dma_start(
                out=wout_t[:].rearrange("p (k c) -> p k c", k=NDC),
                in_=wout.rearrange("k p c -> p k c"))
            bo_bc_box = []

            def load_bo():
                bo_row = mp.tile([1, D], f32, tag="bo_row")
                nc.scalar.dma_start(
                    out=bo_row[:], in_=bo.rearrange("(o f) -> o f", o=1))
                t = mp.tile([KC, D], f32, tag="bo_bc")
                nc.gpsimd.partition_broadcast(out_ap=t[:], in_ap=bo_row[:])
                bo_bc_box.append(t)

            # ---- persistent activations -----------------------------------
            # QQ[p][s]: rows 0:64 = Q^T head 2c, 64:128 = head 2c+1 (batch p)
            QQ = [[qkvp.tile([KC, SB], bf16, tag=f"QQ{p}_{s}", name=f"QQ{p}_{s}")
                   for s in range(NSB)] for p in range(B)]
            KK = [[qkvp.tile([KC, SB], bf16, tag=f"KK{p}_{s}", name=f"KK{p}_{s}")
                   for s in range(NSB)] for p in range(B)]
            # V[2p+hh][s]: [128, 4*65]; chunk sc at cols 65sc..+64, col 65sc+64=1
            NCS = SB // KC
            V = [[qkvp.tile([KC, NCS * (DH + 1)], bf16, tag=f"V{v}_{s}",
                            name=f"V{v}_{s}")
                  for s in range(NSB)] for v in range(2 * B)]
            for v in range(2 * B):
                for s in range(NSB):
                    vv = V[v][s][:].rearrange("p (k c) -> p k c", c=DH + 1)
                    nc.vector.tensor_copy(
                        vv[:, :, DH], vones_sb[:, s * NCS:(s + 1) * NCS])
            # O[p]: rows 0:64 = head 2c out^T (normalized), 64:128 = head 2c+1
            O = [op.tile([KC, S], bf16, tag=f"O{p}", name=f"O{p}")
                 for p in range(B)]

            # P score tiles (bf16, post-exp).  Tag per k-chunk; low chunks
            # are double-buffered since they recur every quarter.
            def p_tile(p, q, kc):
                return pb.tile([KC, 2, SB], bf16, tag=f"P{kc}",
                               name=f"P{p}_{q}_{kc}",
                               bufs=(2 if kc < 8 else 1))

            # ---------------------------------------------------------------
            def proj(sblk):
                vts = []
                for bb in range(B):
                    xs = []
                    for j in range(NDC // 2):
                        xtl = xp.tile([KC, 2 * SB], bf16, tag="xt")
                        eng = (nc.sync, nc.gpsimd, nc.sync, nc.gpsimd)[j]
                        eng.dma_start(
                            out=xtl[:].rearrange("p (k t) -> p k t", k=2),
                            in_=xt[bb, sblk, 2 * j:2 * j + 2].rearrange(
                                "k p t -> p k t"),
                        )
                        xs.append(xtl)

                    def xchunk(k):
                        return xs[k // 2][:, (k % 2) * SB:(k % 2 + 1) * SB]

                    # m = 0 -> Q^T pair, m = 1 -> K^T pair
                    for m in range(2):
                        ps = pp.tile([KC, SB], f32, tag="ps_qk", bufs=1)
                        for k in range(NDC):
                            nc.tensor.matmul(
                                ps[:],
                                wqk_t[:, (2 * k + m) * KC:(2 * k + m + 1) * KC],
                                xchunk(k),
                                start=(k == 0),
                                stop=(k == NDC - 1),
                            )
                        dest = (QQ if m == 0 else KK)[bb][sblk]
                        nc.vector.tensor_scalar_add(dest[:], ps[:], bqk_t[m][:])
                    # V^T: long moving dim, then flip via PE transposes below
                    ps = pp.tile([KC, SB], f32, tag="ps_qk", bufs=1)
                    for k in range(NDC):
                        nc.tensor.matmul(
                            ps[:],
                            wv_t[:, k * KC:(k + 1) * KC],
                            xchunk(k),
                            start=(k == 0),
                            stop=(k == NDC - 1),
                        )
                    vt = ep.tile([KC, SB], bf16, tag="vt", bufs=2,
                                 name=f"vt{bb}_{sblk}")
                    nc.vector.tensor_scalar_add(vt[:], ps[:], bv_t[:])
                    vts.append(vt)
                for bb in range(B):
                    pst = pp.tile([KC, SB], bf16, tag="ps_tr", bufs=1)
                    for sc in range(NCS):
                        nc.tensor.transpose(
                            pst[:, sc * KC:(sc + 1) * KC],
                            vts[bb][:, sc * KC:(sc + 1) * KC],
                            ident_t[:],
                        )
                    ps4 = pst[:].rearrange("p (k h c) -> p k h c", k=NCS, h=2)
                    for hh in range(2):
                        nc.vector.tensor_copy(
                            V[2 * bb + hh][sblk][:].rearrange(
                                "p (k c) -> p k c", c=DH + 1)[:, :, 0:DH],
                            ps4[:, :, hh, :],
                        )

            # ---------------------------------------------------------------
            def attention(qblk):
                nkc = 4 * (qblk + 1)
                for p in range(B):
                    pos = [pp.tile([DH + 1, SB], f32, tag=f"ps_av{hh}",
                                   bufs=1, name=f"po{hh}_{p}_{qblk}")
                           for hh in range(2)]
                    P = [None] * nkc

                    def scores(kc):
                        d = kc - 4 * qblk
                        c0 = KC * max(d, 0)
                        ps = pp.tile([KC, 2, SB], f32, tag="ps_s", bufs=2)
                        for hh in range(2):  # row-tiled, stream concurrently
                            r0 = hh * DH
                            nc.tensor.matmul(
                                ps[:, hh, c0:SB],
                                KK[p][kc // 4][r0:r0 + DH,
                                               (kc % 4) * KC:(kc % 4 + 1) * KC],
                                QQ[p][qblk][r0:r0 + DH, c0:SB],
                                start=True,
                                stop=True,
                            )
                        P[kc] = p_tile(p, qblk, kc)
                        nc.scalar.activation(
                            P[kc][:, :, c0:SB],
                            ps[:, :, c0:SB],
                            mybir.ActivationFunctionType.Exp,
                            scale=1.0 / float(np.sqrt(DH)),
                        )
                        if d >= 0:  # diagonal chunk: zero where k > q
                            nc.gpsimd.affine_select(
                                out=P[kc][:, :, c0:SB],
                                in_=P[kc][:, :, c0:SB],
                                pattern=[[0, 2], [1, SB - c0]],
                                compare_op=mybir.AluOpType.is_ge,
                                fill=0.0,
                                base=0,
                                channel_multiplier=-1,
                            )

                    def av(kc):
                        d = kc - 4 * qblk
                        c0 = KC * max(d, 0)
                        for hh in range(2):
                            nc.tensor.matmul(
                                pos[hh][:, c0:SB],
                                V[2 * p + hh][kc // 4][:,
                                    (kc % 4) * (DH + 1):
                                    (kc % 4 + 1) * (DH + 1)],
                                P[kc][:, hh, c0:SB],
                                start=(kc == 0),
                                stop=(kc == nkc - 1),
                            )

                    # interleave: sc(kc) | av(kc-1) keeps ScalarE saturated
                    for kc in range(nkc):
                        scores(kc)
                        if kc >= 1:
                            av(kc - 1)
                    av(nkc - 1)

                    # normalize (PSUM reads stay on DVE; broadcast on GpSimd)
                    den0 = [ep.tile([1, SB], f32, tag=f"den{hh}", bufs=1,
                                    name=f"den{hh}_{p}_{qblk}")
                            for hh in range(2)]
                    rden = [ep.tile([1, SB], f32, tag=f"rden{hh}", bufs=1,
                                    name=f"rden{hh}_{p}_{qblk}")
                            for hh in range(2)]
                    rbc = [ep.tile([DH, SB], f32, tag=f"rbc{hh}", bufs=2,
                                   name=f"rbc{hh}_{p}_{qblk}")
                           for hh in range(2)]
                    for hh in range(2):
                        nc.vector.tensor_copy(den0[hh][:], pos[hh][DH:DH + 1, :])
                    for hh in range(2):
                        nc.vector.reciprocal_approx_fast(
                            rden[hh][:], den0[hh][:])
                    for hh in range(2):
                        nc.gpsimd.partition_broadcast(
                            out_ap=rbc[hh][:], in_ap=rden[hh][:])
                    for hh in range(2):
                        nc.vector.tensor_mul(
                            O[p][hh * DH:hh * DH + DH,
                                 qblk * SB:(qblk + 1) * SB],
                            pos[hh][0:DH, :],
                            rbc[hh][:],
                        )
                    # stage this (batch, quarter) into the sub-A2A buffer
                    nc.sync.dma_start(
                        out=a2a_in[qblk][4 * p:4 * p + 4].rearrange(
                            "t p c -> p t c"),
                        in_=O[p][:, qblk * SB:(qblk + 1) * SB].rearrange(
                            "p (t c) -> p t c", t=4),
                    )
                nc.gpsimd.collective_compute(
                    "AllToAll",
                    mybir.AluOpType.bypass,
                    replica_groups=[[0, 1, 2, 3, 4, 5, 6, 7]],
                    ins=[a2a_in[qblk][:]],
                    outs=[a2a_out[qblk][:]],
                )

            # ---------------------------------------------------------------
            def outproj(qblk):
                recv = []
                for m in range(NDC // 2):
                    rt = rp.tile([KC, 2 * KC], bf16, tag=f"rc{m}",
                                 name=f"rc{m}_{qblk}")
                    eng = nc.sync if m % 2 == 0 else nc.scalar
                    eng.dma_start(
                        out=rt[:].rearrange("p (t c) -> p t c", t=2),
                        in_=a2a_out[qblk][2 * m:2 * m + 2].rearrange(
                            "t p c -> p t c"),
                    )
                    recv.append(rt)
                for nb in range(D // SB):
                    ps = pp.tile([KC, SB], f32, tag="ps_qk", bufs=1)
                    for k in range(NDC):
                        nc.tensor.matmul(
                            ps[:],
                            recv[k // 2][:, (k % 2) * KC:(k % 2 + 1) * KC],
                            wout_t[:, k * D + nb * SB:k * D + (nb + 1) * SB],
                            start=(k == 0),
                            stop=(k == NDC - 1),
                        )
                    ot = ep.tile([KC, SB], f32, tag="osb", bufs=4)
                    nc.vector.tensor_add(
                        ot[:], ps[:], bo_bc_box[0][:, nb * SB:(nb + 1) * SB])
                    nc.sync.dma_start(
                        out=out_ext[qblk][:, nb * SB:(nb + 1) * SB],
                        in_=ot[:],
                    )

            # ---- static schedule ------------------------------------------
            proj(0)
            attention(0)
            proj(1)
            attention(1)
            proj(2)
            attention(2)
            proj(3)
            load_bo()
            attention(3)
            outproj(0)
            outproj(1)
            outproj(2)
            outproj(3)

    nc.compile()
    return nc


def _get_program():
    global _compiled
    if _compiled is None:
        _compiled = _build()
    return _compiled


def _shard_inputs(x, Wqkv, bqkv, Wout, bout):
    """Build the 8 per-core input maps (all host-side numpy, bf16 data)."""
    bf = ml_dtypes.bfloat16
    x = np.asarray(x, dtype=np.float32)
    Wqkv = np.asarray(Wqkv, dtype=np.float32)
    bqkv = np.ascontiguousarray(np.asarray(bqkv, dtype=np.float32))
    Wout = np.asarray(Wout, dtype=np.float32)
    bout = np.ascontiguousarray(np.asarray(bout, dtype=np.float32))

    Wq = Wqkv[:, 0 * D:1 * D]
    Wk = Wqkv[:, 1 * D:2 * D]
    Wv_full = Wqkv[:, 2 * D:3 * D]
    bq = bqkv[0 * D:1 * D]
    bk = bqkv[1 * D:2 * D]
    bv_full = bqkv[2 * D:3 * D]

    # shared across all cores
    xt = np.ascontiguousarray(
        x.transpose(0, 2, 1)                      # [B, D, S]
         .reshape(B, D, NSB, SB).transpose(0, 2, 1, 3)
         .reshape(B, NSB, NDC, KC, SB).astype(bf)
    )
    wout_b = np.ascontiguousarray(Wout.reshape(NDC, KC, D).astype(bf))
    vones = np.ones((KC, NKC), dtype=bf)
    ident = np.eye(KC, dtype=bf)

    in_maps = []
    for c in range(NCORES):
        ha, hb = 2 * c, 2 * c + 1
        wqk_c = np.ascontiguousarray(np.concatenate(
            [Wq[:, ha * DH:(ha + 1) * DH], Wq[:, hb * DH:(hb + 1) * DH],
             Wk[:, ha * DH:(ha + 1) * DH], Wk[:, hb * DH:(hb + 1) * DH]],
            axis=1).reshape(NDC, KC, 2 * KC).astype(bf))
        bqk_c = np.ascontiguousarray(np.concatenate(
            [bq[ha * DH:(ha + 1) * DH], bq[hb * DH:(hb + 1) * DH],
             bk[ha * DH:(ha + 1) * DH], bk[hb * DH:(hb + 1) * DH]]))
        wv_c = np.ascontiguousarray(np.concatenate(
            [Wv_full[:, ha * DH:(ha + 1) * DH],
             Wv_full[:, hb * DH:(hb + 1) * DH]],
            axis=1).reshape(NDC, KC, KC).astype(bf))
        bv_c = np.ascontiguousarray(np.concatenate(
            [bv_full[ha * DH:(ha + 1) * DH], bv_full[hb * DH:(hb + 1) * DH]]))
        in_maps.append({
            "xt": xt, "wqk": wqk_c, "wv": wv_c, "wout": wout_b,
            "bqk": bqk_c, "bv": bv_c, "bo": bout, "vones": vones,
            "ident": ident,
        })
    return in_maps


def run(inputs, trace=False, trace_kwargs=None):
    nc = _get_program()
    in_maps = _shard_inputs(**inputs)
    res = run_bass_kernel_spmd(
        nc, in_maps, list(range(NCORES)), trace=trace,
        **(trace_kwargs or {}),
    )
    out = np.empty((B, S, D), dtype=np.float32)
    for c in range(NCORES):
        b = c // 4
        t4 = c % 4
        oc = res.results[c]["out"]  # [NSB, KC, D]
        for q in range(NSB):
            out[b, SB * q + KC * t4: SB * q + KC * (t4 + 1), :] = oc[q]
    return out, res


def kernel(**inputs):
    out, _ = run(inputs)
    return out
